# revision 1
# baseline (speedup 1.0000x reference)
"""Trainium2 Bass kernel for nn_MaxPool_730144440853.

Math (per batch b):
    d = einsum("czn,dc->dzn", x[b], W)
    scores[c, n] = sum_z x[b,c,z,n] * d[c,z,n]
    idx[c] = argmax_n scores[c, n]
    out[b, c, :] = x[b, c, :, idx[c]]

Sharding: data-parallel over batch B=8 across the 8 NeuronCores; W replicated.

Device computes, per (b, c) row, the top-8 score values + indices (scores via
float32r matmuls on the PE at full rate). Host then re-scores the <=8
candidates per row in float64 from the original fp32 inputs and picks the
exact argmax, making the result bit-robust to the PE's reduced-precision
float32r mode (for this problem the top-8 always contains the true argmax by
a wide margin).
"""

import sys

sys.path.insert(0, "/opt/trn_rl_repo")

import numpy as np

B, C, Z, N = 8, 256, 3, 8192
H = C // 128  # partition halves (2)
T = 512  # n-tile width
NT = N // T

_cache = {}


def _split_multiwait_bir(bir_json: bytes) -> bytes:
    """walrus in this toolchain rejects instructions carrying more than one
    semaphore wait ("Too many sync wait commands"). Rewrite the BIR so any
    instruction with >1 on_wait keeps only the last one; the others are
    hoisted into single-wait EventSemaphore instructions inserted just
    before it on the same engine (engine program order makes this
    equivalent)."""
    import json

    d = json.loads(bir_json)
    n_new = 0
    for fn in d.get("functions", []):
        for blk in fn.get("blocks", []):
            insts = blk.get("instructions", [])
            out = []
            for ins in insts:
                si = ins.get("sync_info")
                waits = si.get("on_wait") if si else None
                if waits and len(waits) > 1:
                    for w in waits[:-1]:
                        out.append(
                            {
                                "debug": ins.get("debug", 0),
                                "engine": ins["engine"],
                                "ins": [],
                                "name": f"{ins['name']}_hw{n_new}",
                                "opcode": "EventSemaphore",
                                "outs": [],
                                "sync_info": {"on_update": [], "on_wait": [w]},
                            }
                        )
                        n_new += 1
                    si["on_wait"] = [waits[-1]]
                out.append(ins)
            blk["instructions"] = out
    return json.dumps(d).encode()


def _apply_tile_patch():
    """Install the multi-wait splitter in front of walrus compilation."""
    from concourse import bass_utils, bass2jax

    if getattr(bass_utils, "_ant_split_multiwait", False):
        return

    orig = bass_utils.compile_bir_kernel

    def patched(bir_json, tmpdir, neff_name="file.neff"):
        return orig(_split_multiwait_bir(bir_json), tmpdir, neff_name=neff_name)

    bass_utils.compile_bir_kernel = patched
    bass2jax.compile_bir_kernel = patched
    bass_utils._ant_split_multiwait = True


def _build_nc(work_mult=1):
    import concourse.bass as bass
    import concourse.mybir as mybir
    from concourse.tile import TileContext

    _apply_tile_patch()

    f32 = mybir.dt.float32
    f32r = mybir.dt.float32r
    add = mybir.AluOpType.add
    mult = mybir.AluOpType.mult

    nc = bass.Bass(target_bir_lowering=False)
    x0 = nc.dram_tensor("x0", [128, Z, N], f32r, kind="ExternalInput")
    x1 = nc.dram_tensor("x1", [128, Z, N], f32r, kind="ExternalInput")
    # wt{k}[c_in, c_out] = W[c_out, c_in] slices; lhsT for the PE.
    wt0 = nc.dram_tensor("wt0", [128, C], f32r, kind="ExternalInput")
    wt1 = nc.dram_tensor("wt1", [128, C], f32r, kind="ExternalInput")
    v8 = nc.dram_tensor("v8", [H, 128, 8], f32, kind="ExternalOutput")
    i8 = nc.dram_tensor("i8", [H, 128, 8], mybir.dt.uint32, kind="ExternalOutput")

    with TileContext(nc) as tc:
        with (
            tc.tile_pool(name="wts", bufs=1) as wpool,
            tc.tile_pool(name="scores", bufs=1) as spool,
            tc.tile_pool(name="xin", bufs=3) as xpool,
            tc.tile_pool(name="prod", bufs=3) as ppool,
            tc.tile_pool(name="psum", bufs=2, space="PSUM") as dpool,
            tc.tile_pool(name="outs", bufs=1) as opool,
        ):
            wt_sb = []
            for k, wt in enumerate((wt0, wt1)):
                w = wpool.tile([128, C], f32r, tag=f"wt{k}", name=f"wt_sb{k}")
                nc.sync.dma_start(out=w[:], in_=wt[:])
                wt_sb.append(w)

            sc = [spool.tile([128, N], f32, tag=f"sc{h}", name=f"sc{h}") for h in range(H)]

            for rep in range(work_mult):
                for t in range(NT):
                    xt = []
                    for k, xsrc in enumerate((x0, x1)):
                        xk = xpool.tile([128, Z, T], f32r, tag=f"x{k}", name=f"xt{k}")
                        nc.sync.dma_start(
                            out=xk[:], in_=xsrc[:, :, t * T : (t + 1) * T]
                        )
                        xt.append(xk)
                    for h in range(H):
                        d = dpool.tile([128, Z, T], f32, name="d_psum")
                        for z in range(Z):
                            nc.tensor.matmul(
                                d[:, z, :],
                                wt_sb[0][:, h * 128 : (h + 1) * 128],
                                xt[0][:, z, :],
                                start=True,
                                stop=False,
                            )
                            nc.tensor.matmul(
                                d[:, z, :],
                                wt_sb[1][:, h * 128 : (h + 1) * 128],
                                xt[1][:, z, :],
                                start=False,
                                stop=True,
                            )
                        p = ppool.tile([128, Z, T], f32, tag="p", name="p")
                        nc.vector.tensor_tensor(p[:], xt[h][:].bitcast(f32), d[:], op=mult)
                        s1 = ppool.tile([128, T], f32, tag="s1", name="s1")
                        nc.gpsimd.tensor_tensor(s1[:], p[:, 0, :], p[:, 1, :], op=add)
                        nc.vector.tensor_tensor(
                            sc[h][:, t * T : (t + 1) * T], s1[:], p[:, 2, :], op=add
                        )

            for h in range(H):
                vt = opool.tile([128, 8], f32, tag=f"v{h}", name=f"vt{h}")
                it = opool.tile([128, 8], mybir.dt.uint32, tag=f"i{h}", name=f"it{h}")
                nc.vector.max(vt[:], sc[h][:])
                nc.vector.max_index(it[:], vt[:], sc[h][:])
                nc.sync.dma_start(out=v8[h], in_=vt[:])
                nc.sync.dma_start(out=i8[h], in_=it[:])

    return nc


def _get_nc():
    if "nc" not in _cache:
        _cache["nc"] = _build_nc()
    return _cache["nc"]


def _run_device(x, W):
    from concourse.bass_utils import run_bass_kernel_spmd

    nc = _get_nc()
    wt = np.ascontiguousarray(W.T).astype(np.float32)
    in_maps = []
    for b in range(B):
        in_maps.append(
            {
                "x0": np.ascontiguousarray(x[b, :128]),
                "x1": np.ascontiguousarray(x[b, 128:]),
                "wt0": wt[:128],
                "wt1": wt[128:],
            }
        )
    res = run_bass_kernel_spmd(nc, in_maps, core_ids=list(range(B)))
    v8 = np.stack([r["v8"].reshape(C, 8) for r in res.results])  # [B, C, 8]
    i8 = np.stack([r["i8"].reshape(C, 8) for r in res.results])  # [B, C, 8]
    return v8, i8, res


def _host_finalize(x, W, i8):
    """Exact (float64) re-score of the <=8 device candidates per row, then
    gather the winning 3-vector from the original fp32 x."""
    out = np.empty((B, C, Z), dtype=x.dtype)
    W64 = W.astype(np.float64)
    for b in range(B):
        xb = x[b]  # [C, Z, N] fp32
        I = np.minimum(i8[b].astype(np.int64), N - 1)  # [C, 8]
        xb64 = xb.astype(np.float64)
        # cols[c_in, z, r, j] = x[b, c_in, z, I[r, j]]
        cols = xb64[:, :, I]
        d_cand = np.einsum("rc,czrj->rzj", W64, cols)  # [C, Z, 8]
        xr = np.take_along_axis(xb64, I[:, None, :], axis=2)  # [C, Z, 8]
        s_cand = (xr * d_cand).sum(axis=1)  # [C, 8]
        # argmax over candidates; break exact ties toward the smallest n
        # (matches jnp.argmax first-occurrence semantics).
        order = np.lexsort((I, -s_cand), axis=1)
        jbest = order[:, 0]
        nbest = I[np.arange(C), jbest]
        out[b] = np.take_along_axis(
            xb, nbest[:, None, None], axis=2
        )[:, :, 0]
    return out


def kernel(x, W):
    x = np.asarray(x, dtype=np.float32)
    W = np.asarray(W, dtype=np.float32)
    v8, i8, _ = _run_device(x, W)
    return _host_finalize(x, W, i8)



# revision 4
# speedup vs baseline: 1.4074x; 1.4074x over previous
"""Trainium2 Bass kernel for nn_MaxPool_730144440853.

Math (per batch b):
    d = einsum("czn,dc->dzn", x[b], W)
    scores[c, n] = sum_z x[b,c,z,n] * d[c,z,n]
    idx[c] = argmax_n scores[c, n]
    out[b, c, :] = x[b, c, :, idx[c]]

Sharding: data-parallel over batch B=8 across the 8 NeuronCores; W replicated.

Device pipeline (per core, fp16 inputs):
  - PE: d = W @ x per n-tile, fp16 matmuls into fp32 PSUM ([128, 3*512] free
    dim to amortize weight loads).
  - Act: cast d PSUM fp32 -> SBUF fp16 (unlocks the DVE 2x 16-bit mode).
  - DVE (2x): p = x * d, partial z-adds.
  - Pool: remaining z-add (alternating with DVE) and a running elementwise
    max m[q] = max_t s[t*512+q] folded across the 16 n-tiles.
  - DVE tail: max8 + max_index over the folded [128, 512] max array only.

Device returns, per (b, c) row, the top-8 folded positions q_j. The true
argmax position n* = t*512+q satisfies q in {q_j} unless >=8 distinct folded
positions beat the true maximum within fp16 noise (~0.1% of sigma), which is
astronomically unlikely. Host expands the 8 q's to 8*16=128 candidate n's,
re-scores them exactly in float64 from the original fp32 inputs, and picks
the argmax (ties toward smallest n, matching jnp.argmax first-occurrence).
"""

import sys

sys.path.insert(0, "/opt/trn_rl_repo")

import numpy as np

B, C, Z, N = 8, 256, 3, 8192
H = C // 128  # partition halves (2)
T = 512  # n-tile width
NT = N // T
ZT = Z * T

_cache = {}


def _split_multiwait_bir(bir_json: bytes) -> bytes:
    """walrus in this toolchain rejects instructions carrying more than one
    semaphore wait ("Too many sync wait commands"). Rewrite the BIR so any
    instruction with >1 on_wait keeps only the last one; the others are
    hoisted into single-wait EventSemaphore instructions inserted just
    before it on the same engine (engine program order makes this
    equivalent)."""
    import json

    d = json.loads(bir_json)
    n_new = 0
    for fn in d.get("functions", []):
        for blk in fn.get("blocks", []):
            insts = blk.get("instructions", [])
            out = []
            for ins in insts:
                si = ins.get("sync_info")
                waits = si.get("on_wait") if si else None
                if waits and len(waits) > 1:
                    for w in waits[:-1]:
                        out.append(
                            {
                                "debug": ins.get("debug", 0),
                                "engine": ins["engine"],
                                "ins": [],
                                "name": f"{ins['name']}_hw{n_new}",
                                "opcode": "EventSemaphore",
                                "outs": [],
                                "sync_info": {"on_update": [], "on_wait": [w]},
                            }
                        )
                        n_new += 1
                    si["on_wait"] = [waits[-1]]
                out.append(ins)
            blk["instructions"] = out
    return json.dumps(d).encode()


def _apply_tile_patch():
    """Install the multi-wait splitter in front of walrus compilation."""
    from concourse import bass_utils, bass2jax

    if getattr(bass_utils, "_ant_split_multiwait", False):
        return

    orig = bass_utils.compile_bir_kernel

    def patched(bir_json, tmpdir, neff_name="file.neff"):
        return orig(_split_multiwait_bir(bir_json), tmpdir, neff_name=neff_name)

    bass_utils.compile_bir_kernel = patched
    bass2jax.compile_bir_kernel = patched
    bass_utils._ant_split_multiwait = True


def _build_nc():
    import concourse.bass as bass
    import concourse.mybir as mybir
    from concourse.tile import TileContext

    _apply_tile_patch()

    f16 = mybir.dt.float16
    f32 = mybir.dt.float32
    u32 = mybir.dt.uint32
    add = mybir.AluOpType.add
    mult = mybir.AluOpType.mult
    vmax = mybir.AluOpType.max

    nc = bass.Bass(target_bir_lowering=False)
    # x{k}[t] = fp16 tile [128, Z*T], channels k*128..k*128+127, n-tile t.
    x0 = nc.dram_tensor("x0", [NT, 128, ZT], f16, kind="ExternalInput")
    x1 = nc.dram_tensor("x1", [NT, 128, ZT], f16, kind="ExternalInput")
    # wt{k}[c_in - k*128, c_out] = W[c_out, c_in]; lhsT slices for the PE.
    wt0 = nc.dram_tensor("wt0", [128, C], f16, kind="ExternalInput")
    wt1 = nc.dram_tensor("wt1", [128, C], f16, kind="ExternalInput")
    v8 = nc.dram_tensor("v8", [H, 128, 8], f16, kind="ExternalOutput")
    i8 = nc.dram_tensor("i8", [H, 128, 8], u32, kind="ExternalOutput")

    with TileContext(nc) as tc:
        with (
            tc.tile_pool(name="wts", bufs=1) as wpool,
            tc.tile_pool(name="xin", bufs=3) as xpool,
            tc.tile_pool(name="dcast", bufs=3) as cpool,
            tc.tile_pool(name="prod", bufs=3) as ppool,
            tc.tile_pool(name="sums", bufs=3) as spool,
            tc.tile_pool(name="fold", bufs=2) as mpool,
            tc.tile_pool(name="psum", bufs=2, space="PSUM") as dpool,
            tc.tile_pool(name="outs", bufs=1) as opool,
        ):
            wt_sb = []
            for k, wt in enumerate((wt0, wt1)):
                w = wpool.tile([128, C], f16, tag=f"wt{k}", name=f"wt_sb{k}")
                nc.sync.dma_start(out=w[:], in_=wt[:])
                wt_sb.append(w)

            # running folded max per half, ping-pong buffers via the pool
            m_prev = []
            for h in range(H):
                m0 = mpool.tile([128, T], f16, tag=f"m{h}", name=f"m{h}")
                nc.gpsimd.memset(m0[:], -60000.0)
                m_prev.append(m0)

            for t in range(NT):
                xt = []
                for k, xsrc in enumerate((x0, x1)):
                    xk = xpool.tile([128, ZT], f16, tag=f"x{k}", name=f"xt{k}")
                    nc.sync.dma_start(out=xk[:], in_=xsrc[t])
                    xt.append(xk)
                for h in range(H):
                    d = dpool.tile([128, ZT], f32, name="d_psum")
                    # k-major order: 3 consecutive matmuls share the same
                    # stationary weights (one PSUM bank = 512 fp32 per mm).
                    for k in range(2):
                        for z in range(Z):
                            nc.tensor.matmul(
                                d[:, z * T : (z + 1) * T],
                                wt_sb[k][:, h * 128 : (h + 1) * 128],
                                xt[k][:, z * T : (z + 1) * T],
                                start=(k == 0),
                                stop=(k == 1),
                            )
                    dc = cpool.tile([128, ZT], f16, tag="dc", name="dc")
                    nc.scalar.copy(dc[:], d[:])
                    p = ppool.tile([128, ZT], f16, tag="p", name="p")
                    nc.vector.tensor_tensor(p[:], xt[h][:], dc[:], op=mult)
                    s1 = spool.tile([128, T], f16, tag="s1", name="s1")
                    nc.vector.tensor_tensor(s1[:], p[:, 0:T], p[:, T : 2 * T], op=add)
                    # Pool has no max op, so it takes the second z-add and the
                    # running fold-max stays on DVE (2x fp16 mode).
                    s = spool.tile([128, T], f16, tag="s", name="s")
                    nc.gpsimd.tensor_tensor(s[:], s1[:], p[:, 2 * T : 3 * T], op=add)
                    m_new = mpool.tile([128, T], f16, tag=f"m{h}", name=f"m{h}n")
                    nc.vector.tensor_tensor(m_new[:], m_prev[h][:], s[:], op=vmax)
                    m_prev[h] = m_new

            for h in range(H):
                vt = opool.tile([128, 8], f16, tag=f"v{h}", name=f"vt{h}")
                it = opool.tile([128, 8], u32, tag=f"i{h}", name=f"it{h}")
                nc.vector.max(vt[:], m_prev[h][:])
                nc.vector.max_index(it[:], vt[:], m_prev[h][:])
                nc.sync.dma_start(out=v8[h], in_=vt[:])
                nc.sync.dma_start(out=i8[h], in_=it[:])

    return nc


def _get_nc():
    if "nc" not in _cache:
        _cache["nc"] = _build_nc()
    return _cache["nc"]


def _make_in_maps(x, W):
    """Per-core input dict: fp16 tiled x halves + transposed fp16 W slices."""
    wt = np.ascontiguousarray(W.T).astype(np.float16)
    x16 = x.astype(np.float16)  # [B, C, Z, N]
    in_maps = []
    for b in range(B):
        m = {"wt0": wt[:128], "wt1": wt[128:]}
        for k in range(2):
            # [128, Z, NT, T] -> [NT, 128, Z*T]
            xk = x16[b, k * 128 : (k + 1) * 128].reshape(128, Z, NT, T)
            m[f"x{k}"] = np.ascontiguousarray(xk.transpose(2, 0, 1, 3)).reshape(
                NT, 128, ZT
            )
        in_maps.append(m)
    return in_maps


def _run_device(x, W):
    from concourse.bass_utils import run_bass_kernel_spmd

    nc = _get_nc()
    res = run_bass_kernel_spmd(nc, _make_in_maps(x, W), core_ids=list(range(B)))
    v8 = np.stack([r["v8"].reshape(C, 8) for r in res.results])  # [B, C, 8] f16
    i8 = np.stack([r["i8"].reshape(C, 8) for r in res.results])  # [B, C, 8] u32
    return v8, i8, res


def _host_finalize(x, W, i8):
    """Expand the 8 folded positions per row to 8*NT candidate indices,
    re-score them exactly in float64, and gather the winning 3-vector."""
    out = np.empty((B, C, Z), dtype=x.dtype)
    W64 = W.astype(np.float64)
    offs = (np.arange(NT, dtype=np.int64) * T)[None, :, None]  # [1, NT, 1]
    NC = NT * 8
    for b in range(B):
        xb = x[b]  # [C, Z, N] fp32
        q = np.minimum(i8[b].astype(np.int64), T - 1)  # [C, 8]
        I = (q[:, None, :] + offs).reshape(C, NC)  # [C, NC]
        xb64 = xb.astype(np.float64)
        s_cand = np.empty((C, NC), dtype=np.float64)
        blk = 64
        for r0 in range(0, C, blk):
            r1 = r0 + blk
            # cols[c_in, z, r, j] = x[b, c_in, z, I[r, j]]
            cols = xb64[:, :, I[r0:r1]]  # [C, Z, blk, NC]
            d_cand = np.einsum("rc,czrj->rzj", W64[r0:r1], cols)
            xr = np.take_along_axis(
                xb64[r0:r1], I[r0:r1, None, :], axis=2
            )  # [blk, Z, NC]
            s_cand[r0:r1] = (xr * d_cand).sum(axis=1)
        # argmax over candidates; break exact ties toward the smallest n
        # (matches jnp.argmax first-occurrence semantics).
        order = np.lexsort((I, -s_cand), axis=1)
        jbest = order[:, 0]
        nbest = I[np.arange(C), jbest]
        out[b] = np.take_along_axis(xb, nbest[:, None, None], axis=2)[:, :, 0]
    return out


def kernel(x, W):
    x = np.asarray(x, dtype=np.float32)
    W = np.asarray(W, dtype=np.float32)
    v8, i8, _ = _run_device(x, W)
    return _host_finalize(x, W, i8)


# revision 6
# speedup vs baseline: 1.4309x; 1.0167x over previous
"""Trainium2 Bass kernel for nn_MaxPool_730144440853.

Math (per batch b):
    d = einsum("czn,dc->dzn", x[b], W)
    scores[c, n] = sum_z x[b,c,z,n] * d[c,z,n]
    idx[c] = argmax_n scores[c, n]
    out[b, c, :] = x[b, c, :, idx[c]]

Sharding: data-parallel over batch B=8 across the 8 NeuronCores; W replicated.

Device pipeline (per core, fp16 inputs):
  - PE: d = W @ x per n-tile, fp16 matmuls into fp32 PSUM ([128, 3*512] free
    dim to amortize weight loads).
  - Act: cast d PSUM fp32 -> SBUF fp16 (unlocks the DVE 2x 16-bit mode).
  - DVE (2x): p = x * d, partial z-adds.
  - Pool: remaining z-add (alternating with DVE) and a running elementwise
    max m[q] = max_t s[t*512+q] folded across the 16 n-tiles.
  - DVE tail: max8 + max_index over the folded [128, 512] max array only.

Device returns, per (b, c) row, the top-8 folded positions q_j. The true
argmax position n* = t*512+q satisfies q in {q_j} unless >=8 distinct folded
positions beat the true maximum within fp16 noise (~0.1% of sigma), which is
astronomically unlikely. Host expands the 8 q's to 8*16=128 candidate n's,
re-scores them exactly in float64 from the original fp32 inputs, and picks
the argmax (ties toward smallest n, matching jnp.argmax first-occurrence).
"""

import sys

sys.path.insert(0, "/opt/trn_rl_repo")

import numpy as np

B, C, Z, N = 8, 256, 3, 8192
H = C // 128  # partition halves (2)
T = 512  # n-tile width
NT = N // T
ZT = Z * T

_cache = {}


def _split_multiwait_bir(bir_json: bytes) -> bytes:
    """walrus in this toolchain rejects instructions carrying more than one
    semaphore wait ("Too many sync wait commands"). Rewrite the BIR so any
    instruction with >1 on_wait keeps only the last one; the others are
    hoisted into single-wait EventSemaphore instructions inserted just
    before it on the same engine (engine program order makes this
    equivalent)."""
    import json

    d = json.loads(bir_json)
    n_new = 0
    for fn in d.get("functions", []):
        for blk in fn.get("blocks", []):
            insts = blk.get("instructions", [])
            out = []
            for ins in insts:
                si = ins.get("sync_info")
                waits = si.get("on_wait") if si else None
                if waits and len(waits) > 1:
                    for w in waits[:-1]:
                        out.append(
                            {
                                "debug": ins.get("debug", 0),
                                "engine": ins["engine"],
                                "ins": [],
                                "name": f"{ins['name']}_hw{n_new}",
                                "opcode": "EventSemaphore",
                                "outs": [],
                                "sync_info": {"on_update": [], "on_wait": [w]},
                            }
                        )
                        n_new += 1
                    si["on_wait"] = [waits[-1]]
                out.append(ins)
            blk["instructions"] = out
    return json.dumps(d).encode()


def _apply_tile_patch():
    """Install the multi-wait splitter in front of walrus compilation."""
    from concourse import bass_utils, bass2jax

    if getattr(bass_utils, "_ant_split_multiwait", False):
        return

    orig = bass_utils.compile_bir_kernel

    def patched(bir_json, tmpdir, neff_name="file.neff"):
        return orig(_split_multiwait_bir(bir_json), tmpdir, neff_name=neff_name)

    bass_utils.compile_bir_kernel = patched
    bass2jax.compile_bir_kernel = patched
    bass_utils._ant_split_multiwait = True


def _build_nc():
    import concourse.bass as bass
    import concourse.mybir as mybir
    from concourse.tile import TileContext

    _apply_tile_patch()

    f16 = mybir.dt.float16
    f32 = mybir.dt.float32
    u32 = mybir.dt.uint32
    add = mybir.AluOpType.add
    mult = mybir.AluOpType.mult
    vmax = mybir.AluOpType.max

    nc = bass.Bass(target_bir_lowering=False)
    # x{k}[t] = fp16 tile [128, Z*T], channels k*128..k*128+127, n-tile t.
    x0 = nc.dram_tensor("x0", [NT, 128, ZT], f16, kind="ExternalInput")
    x1 = nc.dram_tensor("x1", [NT, 128, ZT], f16, kind="ExternalInput")
    # wt{k}[c_in - k*128, c_out] = W[c_out, c_in]; lhsT slices for the PE.
    wt0 = nc.dram_tensor("wt0", [128, C], f16, kind="ExternalInput")
    wt1 = nc.dram_tensor("wt1", [128, C], f16, kind="ExternalInput")
    v8 = nc.dram_tensor("v8", [H, 128, 8], f16, kind="ExternalOutput")
    i8 = nc.dram_tensor("i8", [H, 128, 8], u32, kind="ExternalOutput")

    with TileContext(nc) as tc:
        with (
            tc.tile_pool(name="wts", bufs=1) as wpool,
            tc.tile_pool(name="xin", bufs=4) as xpool,
            tc.tile_pool(name="dcast", bufs=4) as cpool,
            tc.tile_pool(name="prod", bufs=4) as ppool,
            tc.tile_pool(name="sums", bufs=6) as spool,
            tc.tile_pool(name="fold", bufs=2) as mpool,
            tc.tile_pool(name="psum", bufs=2, space="PSUM") as dpool,
            tc.tile_pool(name="outs", bufs=1) as opool,
        ):
            wt_sb = []
            for k, wt in enumerate((wt0, wt1)):
                w = wpool.tile([128, C], f16, tag=f"wt{k}", name=f"wt_sb{k}")
                nc.sync.dma_start(out=w[:], in_=wt[:])
                wt_sb.append(w)

            # running folded max per half, ping-pong buffers via the pool
            m_prev = []
            for h in range(H):
                m0 = mpool.tile([128, T], f16, tag=f"m{h}", name=f"m{h}")
                nc.gpsimd.memset(m0[:], -60000.0)
                m_prev.append(m0)

            # Software-pipelined fold: rmax(i) is emitted 2 iterations late so
            # the Pool add2(i) latency hides behind the next tiles' DVE work
            # (engines execute in program order; an early rmax would stall DVE).
            pending = []

            def flush_rmax():
                h_, s_ = pending.pop(0)
                m_new = mpool.tile([128, T], f16, tag=f"m{h_}", name=f"m{h_}n")
                nc.vector.tensor_tensor(m_new[:], m_prev[h_][:], s_[:], op=vmax)
                m_prev[h_] = m_new

            for t in range(NT):
                xt = []
                for k, xsrc in enumerate((x0, x1)):
                    xk = xpool.tile([128, ZT], f16, tag=f"x{k}", name=f"xt{k}")
                    nc.sync.dma_start(out=xk[:], in_=xsrc[t])
                    xt.append(xk)
                for h in range(H):
                    d = dpool.tile([128, ZT], f32, name="d_psum")
                    # k-major order: 3 consecutive matmuls share the same
                    # stationary weights (one PSUM bank = 512 fp32 per mm).
                    for k in range(2):
                        for z in range(Z):
                            nc.tensor.matmul(
                                d[:, z * T : (z + 1) * T],
                                wt_sb[k][:, h * 128 : (h + 1) * 128],
                                xt[k][:, z * T : (z + 1) * T],
                                start=(k == 0),
                                stop=(k == 1),
                            )
                    dc = cpool.tile([128, ZT], f16, tag="dc", name="dc")
                    nc.scalar.copy(dc[:], d[:])
                    p = ppool.tile([128, ZT], f16, tag="p", name="p")
                    nc.vector.tensor_tensor(p[:], xt[h][:], dc[:], op=mult)
                    s1 = spool.tile([128, T], f16, tag="s1", name="s1")
                    nc.vector.tensor_tensor(s1[:], p[:, 0:T], p[:, T : 2 * T], op=add)
                    # Pool has no max op, so it takes the second z-add and the
                    # running fold-max stays on DVE (2x fp16 mode).
                    s = spool.tile([128, T], f16, tag="s", name="s")
                    nc.gpsimd.tensor_tensor(s[:], s1[:], p[:, 2 * T : 3 * T], op=add)
                    pending.append((h, s))
                    if len(pending) > 2:
                        flush_rmax()
            while pending:
                flush_rmax()

            for h in range(H):
                vt = opool.tile([128, 8], f16, tag=f"v{h}", name=f"vt{h}")
                it = opool.tile([128, 8], u32, tag=f"i{h}", name=f"it{h}")
                nc.vector.max(vt[:], m_prev[h][:])
                nc.vector.max_index(it[:], vt[:], m_prev[h][:])
                nc.sync.dma_start(out=v8[h], in_=vt[:])
                nc.sync.dma_start(out=i8[h], in_=it[:])

    return nc


def _get_nc():
    if "nc" not in _cache:
        _cache["nc"] = _build_nc()
    return _cache["nc"]


def _make_in_maps(x, W):
    """Per-core input dict: fp16 tiled x halves + transposed fp16 W slices."""
    wt = np.ascontiguousarray(W.T).astype(np.float16)
    x16 = x.astype(np.float16)  # [B, C, Z, N]
    in_maps = []
    for b in range(B):
        m = {"wt0": wt[:128], "wt1": wt[128:]}
        for k in range(2):
            # [128, Z, NT, T] -> [NT, 128, Z*T]
            xk = x16[b, k * 128 : (k + 1) * 128].reshape(128, Z, NT, T)
            m[f"x{k}"] = np.ascontiguousarray(xk.transpose(2, 0, 1, 3)).reshape(
                NT, 128, ZT
            )
        in_maps.append(m)
    return in_maps


def _run_device(x, W):
    from concourse.bass_utils import run_bass_kernel_spmd

    nc = _get_nc()
    res = run_bass_kernel_spmd(nc, _make_in_maps(x, W), core_ids=list(range(B)))
    v8 = np.stack([r["v8"].reshape(C, 8) for r in res.results])  # [B, C, 8] f16
    i8 = np.stack([r["i8"].reshape(C, 8) for r in res.results])  # [B, C, 8] u32
    return v8, i8, res


def _host_finalize(x, W, i8):
    """Expand the 8 folded positions per row to 8*NT candidate indices,
    re-score them exactly in float64, and gather the winning 3-vector."""
    out = np.empty((B, C, Z), dtype=x.dtype)
    W64 = W.astype(np.float64)
    offs = (np.arange(NT, dtype=np.int64) * T)[None, :, None]  # [1, NT, 1]
    NC = NT * 8
    for b in range(B):
        xb = x[b]  # [C, Z, N] fp32
        q = np.minimum(i8[b].astype(np.int64), T - 1)  # [C, 8]
        I = (q[:, None, :] + offs).reshape(C, NC)  # [C, NC]
        xb64 = xb.astype(np.float64)
        s_cand = np.empty((C, NC), dtype=np.float64)
        blk = 64
        for r0 in range(0, C, blk):
            r1 = r0 + blk
            # cols[c_in, z, r, j] = x[b, c_in, z, I[r, j]]
            cols = xb64[:, :, I[r0:r1]]  # [C, Z, blk, NC]
            d_cand = np.einsum("rc,czrj->rzj", W64[r0:r1], cols)
            xr = np.take_along_axis(
                xb64[r0:r1], I[r0:r1, None, :], axis=2
            )  # [blk, Z, NC]
            s_cand[r0:r1] = (xr * d_cand).sum(axis=1)
        # argmax over candidates; break exact ties toward the smallest n
        # (matches jnp.argmax first-occurrence semantics).
        order = np.lexsort((I, -s_cand), axis=1)
        jbest = order[:, 0]
        nbest = I[np.arange(C), jbest]
        out[b] = np.take_along_axis(xb, nbest[:, None, None], axis=2)[:, :, 0]
    return out


def kernel(x, W):
    x = np.asarray(x, dtype=np.float32)
    W = np.asarray(W, dtype=np.float32)
    v8, i8, _ = _run_device(x, W)
    return _host_finalize(x, W, i8)
